# revision 15
# baseline (speedup 1.0000x reference)
"""Trainium2 Bass kernel for nn_HashFlagEmbedding: blake2b(str(id)) -> scale -> tanh -> L2 norm.

Strategy:
- Host (numpy): decimal-string message build + blake2b init + round 0 folded
  (constant-heavy), shipped to device as 16-bit limbs in uint32.
- Device (8 cores, data-parallel over ids): rounds 1..11 of blake2b in 16-bit
  limb arithmetic on the DVE (fp32-exact adds < 2^24, bitvec shifts/xors),
  finalize h = hh ^ v ^ v8, byte-extract via uint8-view + fused
  tanh(1.5*(b/127.5-1)) on the ACT engine, L2-normalize on DVE, DMA out.
"""

import numpy as np
import concourse.bacc as bacc
from concourse import mybir
from concourse.tile import TileContext
from concourse.bass_utils import run_bass_kernel_spmd

U32, U16, U8, F32 = mybir.dt.uint32, mybir.dt.uint16, mybir.dt.uint8, mybir.dt.float32
Alu = mybir.AluOpType
Act = mybir.ActivationFunctionType

FLAG_DIM = 64
N_CORES = 8
P = 128
CARRY_BIAS = float(2.0**-17 - 0.5)

IV = np.array(
    [0x6A09E667F3BCC908, 0xBB67AE8584CAA73B, 0x3C6EF372FE94F82B, 0xA54FF53A5F1D36F1,
     0x510E527FADE682D1, 0x9B05688C2B3E6C1F, 0x1F83D9ABFB41BD6B, 0x5BE0CD19137E2179],
    dtype=np.uint64,
)

SIGMA = [
    [0, 1, 2, 3, 4, 5, 6, 7, 8, 9, 10, 11, 12, 13, 14, 15],
    [14, 10, 4, 8, 9, 15, 13, 6, 1, 12, 0, 2, 11, 7, 5, 3],
    [11, 8, 12, 0, 5, 2, 15, 13, 10, 14, 3, 6, 7, 1, 9, 4],
    [7, 9, 3, 1, 13, 12, 11, 14, 2, 6, 5, 10, 4, 0, 15, 8],
    [9, 0, 5, 7, 2, 4, 10, 15, 14, 1, 11, 12, 6, 8, 3, 13],
    [2, 12, 6, 10, 0, 11, 8, 3, 4, 13, 7, 5, 15, 14, 1, 9],
    [12, 5, 1, 15, 14, 13, 4, 10, 0, 7, 6, 3, 9, 2, 8, 11],
    [13, 11, 7, 14, 12, 1, 3, 9, 5, 0, 15, 4, 8, 6, 2, 10],
    [6, 15, 14, 9, 11, 3, 0, 8, 12, 2, 13, 7, 1, 4, 10, 5],
    [10, 2, 8, 4, 7, 6, 1, 5, 15, 11, 9, 14, 3, 12, 13, 0],
]

COLS = [(0, 4, 8, 12), (1, 5, 9, 13), (2, 6, 10, 14), (3, 7, 11, 15),
        (0, 5, 10, 15), (1, 6, 11, 12), (2, 7, 8, 13), (3, 4, 9, 14)]


# ---------------------------------------------------------------- host side

def _rotr_np(x, n):
    return (x >> np.uint64(n)) | (x << np.uint64(64 - n))


def _host_msg(ids):
    """ids (int64 <2^31) -> (m0, m1, ndig) as uint64 arrays."""
    i64 = ids.astype(np.int64)
    p10 = 10 ** np.arange(1, 10, dtype=np.int64)
    ndig = 1 + (i64[:, None] >= p10[None, :]).sum(1)
    j = np.arange(10)
    e = ndig[:, None] - 1 - j[None, :]
    pw = 10 ** np.clip(e, 0, 9).astype(np.int64)
    digit = (i64[:, None] // pw) % 10
    msg = np.where(e >= 0, digit + 48, 0).astype(np.uint64)
    sh = (8 * np.arange(8)).astype(np.uint64)
    m0 = np.bitwise_or.reduce(msg[:, 0:8] << sh[None, :], axis=1)
    m1 = msg[:, 8] | (msg[:, 9] << np.uint64(8))
    return m0, m1, ndig.astype(np.uint64)


def _host_round0(m0, m1, ndig):
    """Build the init state and fold round 0 on the host.

    Returns v (list of 16 uint64 arrays) and hh (8 uint64 consts)."""
    N = m0.shape[0]
    hh = IV.copy()
    hh[0] ^= np.uint64(0x01010000 ^ FLAG_DIM)
    v = [np.full(N, hh[i], np.uint64) for i in range(8)] + [
        np.full(N, IV[i], np.uint64) for i in range(8)
    ]
    v[12] = v[12] ^ ndig
    v[14] = v[14] ^ np.uint64(0xFFFFFFFFFFFFFFFF)
    z = np.zeros(N, np.uint64)
    m = [m0, m1] + [z] * 14
    s = SIGMA[0]
    with np.errstate(over="ignore"):
        for ci, (a, b, c, d) in enumerate(COLS):
            x, y = m[s[2 * ci]], m[s[2 * ci + 1]]
            v[a] = v[a] + v[b] + x
            v[d] = _rotr_np(v[d] ^ v[a], 32)
            v[c] = v[c] + v[d]
            v[b] = _rotr_np(v[b] ^ v[c], 24)
            v[a] = v[a] + v[b] + y
            v[d] = _rotr_np(v[d] ^ v[a], 16)
            v[c] = v[c] + v[d]
            v[b] = _rotr_np(v[b] ^ v[c], 63)
    return v, hh


# ---------------------------------------------------------------- device side

class _Emitter:
    def __init__(self, nc, pool, fd, pool_adds=False, act_carry=False):
        self.nc = nc
        self.pool = pool
        self.fd = fd
        self.free = []
        self.nalloc = 0
        self.v = nc.vector
        self.pool_adds = pool_adds
        self.act_carry = act_carry

    # -- tile management (virtual registers over uint32 [128, fd] tiles)
    def alloc(self):
        if self.free:
            return self.free.pop()
        t = self.pool.tile([P, self.fd], U32, tag=f"vr{self.nalloc}",
                           name=f"vr{self.nalloc}")
        self.nalloc += 1
        return t

    def rel(self, t):
        self.free.append(t)

    def relw(self, w):
        for t in w:
            self.free.append(t)

    # -- raw ops
    def tt(self, out, a, b, op):
        self.v.tensor_tensor(out[:], a[:], b[:], op)

    def add(self, out, a, b):
        """Integer-exact add: pool (int path) when enabled, else DVE fp32 path."""
        if self.pool_adds:
            self.nc.gpsimd.tensor_tensor(out[:], a[:], b[:], Alu.add)
        else:
            self.v.tensor_tensor(out[:], a[:], b[:], Alu.add)

    def ts2(self, out, a, s1, s2, op0, op1=Alu.bypass):
        self.v.tensor_scalar(out[:], a[:], s1, s2, op0, op1)

    def carry(self, out, s):
        """out = floor(s / 2^16) for s < 2^18, via round-nearest writeback."""
        if self.act_carry:
            self.nc.scalar.activation(out[:], s[:], Act.Copy,
                                      bias=CARRY_BIAS, scale=2.0**-16)
            return
        v = self.v
        v.add_instruction(
            mybir.InstTensorScalarPtr(
                name=self.nc.get_next_instruction_name(),
                op0=Alu.mult, op1=Alu.add,
                ins=[v.lower_ap(s[:]),
                     mybir.ImmediateValue(dtype=F32, value=2.0**-16),
                     mybir.ImmediateValue(dtype=F32, value=CARRY_BIAS)],
                outs=[v.lower_ap(out[:])]))

    def stt_bv(self, out, in0, imm, in1, op0, op1):
        v = self.v
        v.add_instruction(
            mybir.InstTensorScalarPtr(
                name=self.nc.get_next_instruction_name(),
                is_scalar_tensor_tensor=True,
                op0=op0, op1=op1,
                ins=[v.lower_ap(in0[:]),
                     mybir.ImmediateValue(dtype=U32, value=int(imm)),
                     v.lower_ap(in1[:])],
                outs=[v.lower_ap(out[:])]))

    # -- 64-bit word ops on 4x16-bit limbs
    def add64(self, A, B, X=None, X1=None, mask_top=False):
        """S = A + B (+X word) (+X1 single-limb); releases A. B/X untouched.

        Limb 3 is left unmasked unless mask_top (its overflow is >= bit 64;
        consumers fuse the mask via stt). Stays fp32-exact: growth is bounded
        at ~2 bits per round, well under 2^24 over 11 rounds."""
        S = [self.alloc() for _ in range(4)]
        for i in range(4):
            self.add(S[i], A[i], B[i])
        if X is not None:
            for i in range(4):
                self.add(S[i], S[i], X[i])
        if X1 is not None:
            self.add(S[0], S[0], X1)
        c = self.alloc()
        for i in range(3):
            self.carry(c, S[i])
            self.add(S[i + 1], S[i + 1], c)
            self.ts2(S[i], S[i], 0xFFFF, None, Alu.bitwise_and)
        if mask_top:
            self.ts2(S[3], S[3], 0xFFFF, None, Alu.bitwise_and)
        self.rel(c)
        self.relw(A)
        return S

    def xor64(self, A, B, b_dirty_top=True):
        """D = A ^ B; releases A. B untouched.

        When b_dirty_top, B's limb 3 may carry junk above bit 16: fuse the
        mask into the xor via stt."""
        D = [self.alloc() for _ in range(4)]
        for i in range(3):
            self.tt(D[i], A[i], B[i], Alu.bitwise_xor)
        if b_dirty_top:
            self.stt_bv(D[3], B[3], 0xFFFF, A[3],
                        Alu.bitwise_and, Alu.bitwise_xor)
        else:
            self.tt(D[3], A[3], B[3], Alu.bitwise_xor)
        self.relw(A)
        return D

    @staticmethod
    def rot32(A):
        return [A[2], A[3], A[0], A[1]]

    @staticmethod
    def rot16(A):
        return [A[1], A[2], A[3], A[0]]

    def rotr8(self, A):
        """O_i = (A_i >> 8) | ((A_{i+1} & 0xFF) << 8); releases A."""
        O = [self.alloc() for _ in range(4)]
        t = self.alloc()
        for i in range(4):
            self.ts2(t, A[(i + 1) % 4], 8, 0xFF00, Alu.logical_shift_left,
                     Alu.bitwise_and)
            self.stt_bv(O[i], A[i], 8, t, Alu.logical_shift_right, Alu.bitwise_or)
        self.rel(t)
        self.relw(A)
        return O

    def rot24(self, A):
        return self.rotr8(self.rot16(A))

    def rotl1(self, A):
        """O_i = ((A_i << 1) & 0xFFFE) | (A_{i-1} >> 15); releases A."""
        O = [self.alloc() for _ in range(4)]
        t = self.alloc()
        for i in range(4):
            self.ts2(t, A[i], 1, 0xFFFE, Alu.logical_shift_left, Alu.bitwise_and)
            self.stt_bv(O[i], A[(i - 1) % 4], 15, t, Alu.logical_shift_right,
                        Alu.bitwise_or)
        self.rel(t)
        self.relw(A)
        return O

    def G(self, v, a, b, c, d, x=None, y=None, x1=None, y1=None, mask_top=False):
        A, B, C, D = v[a], v[b], v[c], v[d]
        A = self.add64(A, B, X=x, X1=x1, mask_top=mask_top)
        D = self.rot32(self.xor64(D, A))
        C = self.add64(C, D, mask_top=mask_top)
        B = self.rot24(self.xor64(B, C))
        A = self.add64(A, B, X=y, X1=y1, mask_top=mask_top)
        D = self.rot16(self.xor64(D, A))
        C = self.add64(C, D, mask_top=mask_top)
        B = self.rotl1(self.xor64(B, C))
        v[a], v[b], v[c], v[d] = A, B, C, D


def build_program(fd, nb, n_rounds=12, pool_adds=False, act_carry=False):
    """Build the SPMD program. fd: free-dim per batch; nb: batches.

    DRAM I/O (per core):
      vin  [16, 4, 128, fd*nb] u32 : post-round-0 state limbs
      m0in [4, 128, fd*nb] u32, m1in [128, fd*nb] u32
      out  [128, fd*nb, 64] f32
    """
    fdt = fd * nb
    nc = bacc.Bacc("TRN2", target_bir_lowering=False)

    bias_t = nc.alloc_sbuf_tensor("const-f32-tanh-bias", [P, 1], F32)
    nc.gpsimd.memset(bias_t.ap(), -1.5)
    nc.const_aps.aps[(F32, -1.5)] = bias_t.ap()
    nc.all_engine_barrier()

    vin = nc.dram_tensor("vin", [16, 4, P, fdt], U32, kind="ExternalInput")
    m0in = nc.dram_tensor("m0in", [4, P, fdt], U32, kind="ExternalInput")
    m1in = nc.dram_tensor("m1in", [P, fdt], U32, kind="ExternalInput")
    out = nc.dram_tensor("out", [P, fdt, FLAG_DIM], F32, kind="ExternalOutput")

    hh = IV.copy()
    hh[0] ^= np.uint64(0x01010000 ^ FLAG_DIM)

    # chunking for the finalize stage
    tc_chunk = fd
    while tc_chunk * FLAG_DIM * 4 > 16 * 1024:  # staging <= 16KB/partition
        tc_chunk = (tc_chunk + 1) // 2
    chunks = [(c0, min(tc_chunk, fd - c0)) for c0 in range(0, fd, tc_chunk)]

    with TileContext(nc) as tc:
        with tc.tile_pool(name="pool", bufs=1) as pool:
            em = _Emitter(nc, pool, fd, pool_adds=pool_adds, act_carry=act_carry)
            for b in range(nb):
                sl = slice(b * fd, (b + 1) * fd)
                # load state + message limbs
                v = {}
                for w in range(16):
                    limbs = []
                    for l in range(4):
                        t = em.alloc()
                        nc.sync.dma_start(t[:], vin.ap()[w, l, :, sl])
                        limbs.append(t)
                    v[w] = limbs
                m0 = []
                for l in range(4):
                    t = em.alloc()
                    nc.sync.dma_start(t[:], m0in.ap()[l, :, sl])
                    m0.append(t)
                m1 = em.alloc()
                nc.sync.dma_start(m1[:], m1in.ap()[:, sl])

                # rounds 1..n_rounds-1 (round 0 folded on host)
                for r in range(1, n_rounds):
                    s = SIGMA[r % 10]
                    for ci, (a, bb, cc, dd) in enumerate(COLS):
                        xi, yi = s[2 * ci], s[2 * ci + 1]
                        kw = {}
                        if xi == 0:
                            kw["x"] = m0
                        elif xi == 1:
                            kw["x1"] = m1
                        if yi == 0:
                            kw["y"] = m0
                        elif yi == 1:
                            kw["y1"] = m1
                        em.G(v, a, bb, cc, dd, mask_top=(r == n_rounds - 1), **kw)
                em.relw(m0)
                em.rel(m1)

                # h[w] = hh[w] ^ v[w] ^ v[w+8]
                h = []
                for w in range(8):
                    limbs = []
                    for l in range(4):
                        t = em.alloc()
                        imm = (int(hh[w]) >> (16 * l)) & 0xFFFF
                        em.stt_bv(t, v[w][l], imm, v[w + 8][l],
                                  Alu.bitwise_xor, Alu.bitwise_xor)
                        limbs.append(t)
                    h.append(limbs)
                    em.relw(v[w])
                    em.relw(v[w + 8])

                # finalize per chunk: extract+tanh (ACT), normalize (DVE), DMA out
                for c0, tcw in chunks:
                    stg = pool.tile([P, tcw, FLAG_DIM], F32, tag="stg",
                                    name=f"stg_{b}_{c0}")
                    for w in range(8):
                        for l in range(4):
                            h8 = h[w][l][:].bitcast(U8)
                            src = h8[:, c0 * 4:(c0 + tcw) * 4].rearrange(
                                "p (t x) -> p t x", x=4)[:, :, 0:2]
                            nc.scalar.activation(
                                stg[:, :, w * 8 + l * 2: w * 8 + l * 2 + 2],
                                src, Act.Tanh, bias=-1.5, scale=1.5 / 127.5)
                    sq = pool.tile([P, tcw, FLAG_DIM], F32, tag="sq",
                                   name=f"sq_{b}_{c0}")
                    nc.vector.tensor_tensor(sq[:], stg[:], stg[:], Alu.mult)
                    ss = pool.tile([P, tcw], F32, tag="ss", name=f"ss_{b}_{c0}")
                    nc.vector.tensor_reduce(ss[:], sq[:], mybir.AxisListType.X,
                                            Alu.add)
                    nrm = pool.tile([P, tcw], F32, tag="nrm", name=f"nrm_{b}_{c0}")
                    nc.scalar.activation(nrm[:], ss[:], Act.Sqrt)
                    nc.vector.tensor_scalar(nrm[:], nrm[:], 1e-6, None, Alu.add)
                    rec = pool.tile([P, tcw], F32, tag="rec", name=f"rec_{b}_{c0}")
                    nc.vector.reciprocal(rec[:], nrm[:])
                    nc.vector.tensor_tensor(
                        stg[:], stg[:], rec[:].to_broadcast((P, tcw, FLAG_DIM)),
                        Alu.mult)
                    nc.gpsimd.dma_start(out.ap()[:, b * fd + c0: b * fd + c0 + tcw, :],
                                        stg[:])
                for w in range(8):
                    em.relw(h[w])
    nc.compile()
    return nc


# ---------------------------------------------------------------- entry point

_CACHE = {}
LAST_RES = None


def _get_program(fd, nb, pool_adds=False, act_carry=False):
    key = (fd, nb, pool_adds, act_carry)
    if key not in _CACHE:
        _CACHE[key] = build_program(fd, nb, pool_adds=pool_adds, act_carry=act_carry)
    return _CACHE[key]


def _limbs(x64):
    """uint64 [*] -> uint32 [4, *] 16-bit limbs."""
    return np.stack([(x64 >> np.uint64(16 * l)) & np.uint64(0xFFFF)
                     for l in range(4)]).astype(np.uint32)


def run(ids, fd, nb, pool_adds=False, act_carry=False, **spmd_kwargs):
    global LAST_RES
    ids = np.asarray(ids).astype(np.int64).ravel()
    n = ids.shape[0]
    fdt = fd * nb
    per_core = P * fdt
    total = per_core * N_CORES
    assert n <= total, (n, total)
    pad = np.zeros(total, np.int64)
    pad[:n] = ids

    m0, m1, ndig = _host_msg(pad)
    v, _hh = _host_round0(m0, m1, ndig)

    vin = np.empty((16, 4, total), np.uint32)
    for w in range(16):
        vin[w] = _limbs(v[w])
    m0l = _limbs(m0)
    m1l = (m1 & np.uint64(0xFFFF)).astype(np.uint32)

    nc = _get_program(fd, nb, pool_adds=pool_adds, act_carry=act_carry)
    in_maps = []
    for c in range(N_CORES):
        sl = slice(c * per_core, (c + 1) * per_core)
        in_maps.append({
            "vin": vin[:, :, sl].reshape(16, 4, P, fdt).copy(),
            "m0in": m0l[:, sl].reshape(4, P, fdt).copy(),
            "m1in": m1l[sl].reshape(P, fdt).copy(),
        })
    res = run_bass_kernel_spmd(nc, in_maps, list(range(N_CORES)), **spmd_kwargs)
    LAST_RES = res
    outs = [np.asarray(res.results[c]["out"]).reshape(per_core, FLAG_DIM)
            for c in range(N_CORES)]
    return np.concatenate(outs, axis=0)[:n]


def kernel(ids):
    return run(ids, fd=489, nb=2, act_carry=True)
